# revision 1
# baseline (speedup 1.0000x reference)
"""GQA attention kernel for 8 TRN2 NeuronCores (Bass/Tile).

Problem: h[2,2048,1024] -> out[2,2048,1024]
  q = h @ wq_w.T + wq_b   (16 heads x 64)
  k/v = h @ w{k,v}_w.T + b (4 KV groups x 64, each serves 4 consecutive heads)
  out = softmax(q k^T / 8) v

Sharding: 8 cores = 2 batches x 4 KV groups. Each core computes one
(batch, group): 4 query heads sharing one K/V group. Fully independent,
no collectives. Host pre-transposes h and weight slices (layout prep,
like sharding) so all on-chip matmuls are feature-major.

Per-core dataflow (all seq=2048, d_model=1024, dh=64):
  hT [1024,2048] --PE f32r--> qT[256,2048], kT[64,2048], vT[64,2048]
  vT --PE transpose--> Vtil [16 x (128,65)] bf16 with ones column
  per head: scoresT[k,q] = kT_chunk^T@qT_h (f32r), exp on ACT -> bf16 A^T,
  O^T[65,q] += Vtil^T @ A^T  (row 64 = softmax denominators, ones trick)
  PE-transpose O^T back, DVE reciprocal+scale -> out tile, DMA out.
"""

import sys

for p in ("/opt/pypackages", "/opt/trn_rl_repo"):
    if p not in sys.path:
        sys.path.insert(0, p)

from contextlib import ExitStack

import numpy as np

import concourse.bass as bass
import concourse.mybir as mybir
import concourse.tile as tile
from concourse import bacc
from concourse.bass_utils import run_bass_kernel_spmd
from concourse.masks import make_identity

F32 = mybir.dt.float32
F32R = mybir.dt.float32r
BF16 = mybir.dt.bfloat16

D_MODEL = 1024
SEQ = 2048
N_HEADS_LOCAL = 4   # heads per core (one KV group)
DH = 64
QDIM = N_HEADS_LOCAL * DH  # 256
BS = 2
NG = 4

# dtype knobs
AV_DT = BF16  # dtype of A (exp output) and V in the O matmul


def _r(ap):
    """View an f32 AP as float32r for full-rate PE matmul."""
    return ap.bitcast(F32R)


def build_program():
    nc = bacc.Bacc("TRN2", target_bir_lowering=False, debug=False)

    hT_d = nc.dram_tensor("hT", [D_MODEL, SEQ], F32R, kind="ExternalInput").ap()
    wqT_d = nc.dram_tensor("wqT", [D_MODEL, QDIM], F32R, kind="ExternalInput").ap()
    wkT_d = nc.dram_tensor("wkT", [D_MODEL, DH], F32R, kind="ExternalInput").ap()
    wvT_d = nc.dram_tensor("wvT", [D_MODEL, DH], F32R, kind="ExternalInput").ap()
    bq_d = nc.dram_tensor("bq", [QDIM, 1], F32, kind="ExternalInput").ap()
    bk_d = nc.dram_tensor("bk", [DH, 1], F32, kind="ExternalInput").ap()
    bv_d = nc.dram_tensor("bv", [DH, 1], F32, kind="ExternalInput").ap()
    out_d = nc.dram_tensor("out", [SEQ, QDIM], F32, kind="ExternalOutput").ap()

    ND = D_MODEL // 128  # 8 d-chunks
    NS = SEQ // 128      # 16 seq chunks

    with tile.TileContext(nc) as tc, ExitStack() as ctx:
        sb = ctx.enter_context(tc.tile_pool(name="sb", bufs=1))

        # persistent SBUF tiles
        hT = [sb.tile([128, SEQ], F32R, tag=f"hT{d}", name=f"hT{d}") for d in range(ND)]
        wqT = [sb.tile([128, QDIM], F32R, tag=f"wqT{d}", name=f"wqT{d}") for d in range(ND)]
        wkT = [sb.tile([128, DH], F32R, tag=f"wkT{d}", name=f"wkT{d}") for d in range(ND)]
        wvT = [sb.tile([128, DH], F32R, tag=f"wvT{d}", name=f"wvT{d}") for d in range(ND)]
        bq0 = sb.tile([128, 1], F32, tag="bq0", name="bq0")
        bq1 = sb.tile([128, 1], F32, tag="bq1", name="bq1")
        bkt = sb.tile([DH, 1], F32, tag="bkt", name="bkt")
        bvt = sb.tile([DH, 1], F32, tag="bvt", name="bvt")
        qt0 = sb.tile([128, SEQ], F32R, tag="qt0", name="qt0")   # heads 0,1 (feature-major)
        qt1 = sb.tile([128, SEQ], F32R, tag="qt1", name="qt1")   # heads 2,3
        qh1 = sb.tile([DH, SEQ], F32R, tag="qh1", name="qh1")    # head 1 shifted to partitions 0-63
        qh3 = sb.tile([DH, SEQ], F32R, tag="qh3", name="qh3")
        kT = sb.tile([DH, SEQ], F32R, tag="kT", name="kT")
        vT = sb.tile([DH, SEQ], F32, tag="vT", name="vT")
        vv = [sb.tile([128, DH + 1], AV_DT, tag=f"vv{i}", name=f"vv{i}") for i in range(NS)]
        ident = sb.tile([128, 128], F32, tag="ident", name="ident")
        out_t = [sb.tile([128, QDIM], F32, tag=f"ot{i}", name=f"ot{i}") for i in range(NS)]

        # input DMAs (weights/biases first, then h)
        for d in range(ND):
            nc.sync.dma_start(wqT[d][:, :], wqT_d[d * 128:(d + 1) * 128, :])
            nc.sync.dma_start(wkT[d][:, :], wkT_d[d * 128:(d + 1) * 128, :])
            nc.sync.dma_start(wvT[d][:, :], wvT_d[d * 128:(d + 1) * 128, :])
        nc.sync.dma_start(bq0[:, :], bq_d[0:128, :])
        nc.sync.dma_start(bq1[:, :], bq_d[128:256, :])
        nc.sync.dma_start(bkt[:, :], bk_d[:, :])
        nc.sync.dma_start(bvt[:, :], bv_d[:, :])
        make_identity(nc, ident[:, :])
        for d in range(ND):
            nc.sync.dma_start(hT[d][:, :], hT_d[d * 128:(d + 1) * 128, :])

        # ---- projections: d-chunk outer so PE chases the h DMA ----
        # psum: 8 banks = 2 seq-chunks x 4 targets x [*,512]
        with tc.tile_pool(name="pp", bufs=1, space="PSUM") as pp:
            for half in range(2):  # seq halves of 1024
                pt = {}
                for sq in range(2):
                    pt[sq, 0] = pp.tile([128, 512], F32, tag=f"pp{sq}q0", name=f"pp{sq}q0")
                    pt[sq, 1] = pp.tile([128, 512], F32, tag=f"pp{sq}q1", name=f"pp{sq}q1")
                    pt[sq, 2] = pp.tile([DH, 512], F32, tag=f"pp{sq}k", name=f"pp{sq}k")
                    pt[sq, 3] = pp.tile([DH, 512], F32, tag=f"pp{sq}v", name=f"pp{sq}v")
                for d in range(ND):
                    for sq in range(2):
                        n0 = half * 1024 + sq * 512
                        rhs = hT[d][:, n0:n0 + 512]
                        st = dict(start=(d == 0), stop=(d == ND - 1))
                        nc.tensor.matmul(pt[sq, 0][:, :], wqT[d][:, 0:128], rhs, **st)
                        nc.tensor.matmul(pt[sq, 1][:, :], wqT[d][:, 128:256], rhs, **st)
                        nc.tensor.matmul(pt[sq, 2][:, :], wkT[d][:, :], rhs, **st)
                        nc.tensor.matmul(pt[sq, 3][:, :], wvT[d][:, :], rhs, **st)
                for sq in range(2):
                    n0 = half * 1024 + sq * 512
                    nc.vector.tensor_scalar_add(qt0[:, n0:n0 + 512], pt[sq, 0][:, :], bq0[:, :])
                    nc.vector.tensor_scalar_add(qt1[:, n0:n0 + 512], pt[sq, 1][:, :], bq1[:, :])
                    nc.vector.tensor_scalar_add(kT[:, n0:n0 + 512], pt[sq, 2][:, :], bkt[:, :])
                    nc.vector.tensor_scalar_add(vT[:, n0:n0 + 512], pt[sq, 3][:, :], bvt[:, :])

        # shift heads 1,3 down to partitions 0-63 (SBUF->SBUF DMA)
        nc.sync.dma_start(qh1[:, :], qt0[64:128, :])
        nc.sync.dma_start(qh3[:, :], qt1[64:128, :])

        # V natural layout with ones column: Vtil[i] = [V_chunk | 1] (bf16)
        with tc.tile_pool(name="ptv", bufs=2, space="PSUM") as ptv:
            for i in range(NS):
                p = ptv.tile([128, DH], F32, tag="ptv", name="ptv")
                nc.tensor.transpose(p[:, :], vT[:, i * 128:(i + 1) * 128], ident[0:DH, 0:DH])
                nc.vector.tensor_copy(vv[i][:, 0:DH], p[:, :])
                nc.vector.memset(vv[i][:, DH:DH + 1], 1.0)

        # ---- attention ----
        qviews = [qt0[0:DH, :], qh1[:, :], qt1[0:DH, :], qh3[:, :]]
        with tc.tile_pool(name="psc", bufs=2, space="PSUM") as psc, \
             tc.tile_pool(name="po", bufs=2, space="PSUM") as pop, \
             tc.tile_pool(name="at", bufs=6) as atp, \
             tc.tile_pool(name="ots", bufs=4) as otp, \
             tc.tile_pool(name="rcp", bufs=4) as rcp:
            for l in range(N_HEADS_LOCAL):
                ot = otp.tile([DH + 1, SEQ], F32, tag="ots", name="ots")
                for qh in range(2):  # q halves of 1024
                    po = pop.tile([DH + 1, 1024], F32, tag="po", name="po")
                    for kc in range(NS):
                        ps = psc.tile([128, 1024], F32, tag="ps", name="ps")
                        for n in range(2):
                            nc.tensor.matmul(
                                ps[:, n * 512:(n + 1) * 512],
                                kT[:, kc * 128:(kc + 1) * 128],
                                qviews[l][:, qh * 1024 + n * 512: qh * 1024 + (n + 1) * 512],
                                start=True, stop=True)
                        at = atp.tile([128, 1024], AV_DT, tag="at", name="at")
                        nc.scalar.activation(at[:, :], ps[:, :],
                                             mybir.ActivationFunctionType.Exp)
                        for n in range(2):
                            nc.tensor.matmul(
                                po[:, n * 512:(n + 1) * 512],
                                vv[kc][:, :], at[:, n * 512:(n + 1) * 512],
                                start=(kc == 0), stop=(kc == NS - 1))
                    nc.vector.tensor_copy(ot[:, qh * 1024:(qh + 1) * 1024], po[:, :])
                # tail: transpose back, normalize
                for qc in range(NS):
                    tp = psc.tile([128, DH + 1], F32, tag="ps", name="ps")
                    nc.tensor.transpose(tp[:, :], ot[:, qc * 128:(qc + 1) * 128],
                                        ident[0:DH + 1, 0:DH + 1])
                    rc = rcp.tile([128, 1], F32, tag="rc", name="rc")
                    nc.vector.reciprocal(rc[:, :], tp[:, DH:DH + 1])
                    nc.vector.tensor_scalar_mul(
                        out_t[qc][:, l * DH:(l + 1) * DH], tp[:, 0:DH], rc[:, :])

        for i in range(NS):
            nc.sync.dma_start(out_d[i * 128:(i + 1) * 128, :], out_t[i][:, :])

    nc.compile()
    return nc


_NC = None
LAST_RESULTS = None
LAST_IN_MAPS = None


def kernel(h, wq_w, wq_b, wk_w, wk_b, wv_w, wv_b, **kw):
    global _NC, LAST_RESULTS, LAST_IN_MAPS
    if _NC is None:
        _NC = build_program()

    h = np.asarray(h, np.float32)
    wq_w = np.asarray(wq_w, np.float32)
    wq_b = np.asarray(wq_b, np.float32)
    wk_w = np.asarray(wk_w, np.float32)
    wk_b = np.asarray(wk_b, np.float32)
    wv_w = np.asarray(wv_w, np.float32)
    wv_b = np.asarray(wv_b, np.float32)

    in_maps = []
    for core in range(8):
        b, g = divmod(core, NG)
        # fold the 1/sqrt(dh) score scale into wq/bq
        wq_s = wq_w[g * QDIM:(g + 1) * QDIM, :] * 0.125
        bq_s = wq_b[g * QDIM:(g + 1) * QDIM] * 0.125
        in_maps.append({
            "hT": np.ascontiguousarray(h[b].T),
            "wqT": np.ascontiguousarray(wq_s.T),
            "wkT": np.ascontiguousarray(wk_w[g * DH:(g + 1) * DH, :].T),
            "wvT": np.ascontiguousarray(wv_w[g * DH:(g + 1) * DH, :].T),
            "bq": np.ascontiguousarray(bq_s.reshape(QDIM, 1)),
            "bk": np.ascontiguousarray(wk_b[g * DH:(g + 1) * DH].reshape(DH, 1)),
            "bv": np.ascontiguousarray(wv_b[g * DH:(g + 1) * DH].reshape(DH, 1)),
        })

    res = run_bass_kernel_spmd(_NC, in_maps, core_ids=list(range(8)))
    LAST_RESULTS = res
    LAST_IN_MAPS = in_maps

    out = np.empty((BS, SEQ, 1024), np.float32)
    for core in range(8):
        b, g = divmod(core, NG)
        out[b, :, g * QDIM:(g + 1) * QDIM] = res.results[core]["out"]
    return out


def bench_exec_ns(reps=8, iters=4):
    """Per-NEFF-execution time: chain `reps` executions (data-dependent)
    inside one jitted launch with device-resident inputs; subtract a
    reps=1 launch and divide. Returns ns."""
    import time

    import jax
    from jax.sharding import Mesh, NamedSharding, PartitionSpec
    from jax.experimental.shard_map import shard_map

    from concourse import bass2jax, mybir as _mb

    assert _NC is not None and LAST_IN_MAPS is not None, "call kernel() first"
    nc = _NC
    bass2jax.install_neuronx_cc_hook()
    partition_name = (nc.partition_id_tensor.name
                      if nc.partition_id_tensor else None)

    in_names, out_names, out_avals, zero_outs = [], [], [], []
    for alloc in nc.m.functions[0].allocations:
        if not isinstance(alloc, _mb.MemoryLocationSet):
            continue
        name = alloc.memorylocations[0].name
        if alloc.kind == "ExternalInput":
            if name != partition_name:
                in_names.append(name)
        elif alloc.kind == "ExternalOutput":
            out_names.append(name)
            shape = tuple(alloc.tensor_shape)
            dtype = _mb.dt.np(alloc.dtype)
            out_avals.append(jax.core.ShapedArray(shape, dtype))
            zero_outs.append(np.zeros(shape, dtype))
    n_params = len(in_names)
    all_in_names = in_names + out_names
    if partition_name is not None:
        all_in_names.append(partition_name)

    def _body(*args):
        ins = list(args[:n_params])
        outs = list(args[n_params:])
        pid = ([bass2jax.partition_id_tensor()]
               if partition_name is not None else [])
        outs = list(bass2jax._bass_exec_p.bind(
            *ins, *outs, *pid,
            out_avals=tuple(out_avals),
            in_names=tuple(all_in_names),
            out_names=tuple(out_names),
            lowering_input_output_aliases=(),
            sim_require_finite=True,
            sim_require_nnan=True,
            nc=nc,
        ))
        return tuple(outs)

    devices = jax.devices()[:8]
    mesh = Mesh(np.asarray(devices), ("core",))
    spec = PartitionSpec("core")
    n_outs = len(out_names)
    concat_in = [
        np.concatenate([np.asarray(m[name]) for m in LAST_IN_MAPS], axis=0)
        for name in in_names
    ]
    concat_zeros = [np.zeros((8 * z.shape[0], *z.shape[1:]), z.dtype)
                    for z in zero_outs]
    sh = NamedSharding(mesh, spec)
    dev_args = [jax.device_put(a, sh) for a in concat_in + concat_zeros]

    fn = jax.jit(shard_map(_body, mesh=mesh,
                           in_specs=(spec,) * (n_params + n_outs),
                           out_specs=(spec,) * n_outs, check_rep=False))
    r = fn(*dev_args)  # compile + warm
    jax.block_until_ready(r)

    # stream `reps` async dispatches; device queue serializes executions
    times = {}
    for n in (1, reps):
        best = float("inf")
        for _ in range(iters):
            t0 = time.perf_counter()
            rs = [fn(*dev_args) for _ in range(n)]
            jax.block_until_ready(rs)
            best = min(best, time.perf_counter() - t0)
        times[n] = best
    per_exec = (times[reps] - times[1]) / (reps - 1)
    return per_exec * 1e9, times

